# revision 15
# baseline (speedup 1.0000x reference)
"""Truncated-signature kernel (CLF_Adam_Layer) for 8x TRN2 NeuronCores.

Input  x: [8, 32, 64] fp32.  Per (batch, segment): v = -x[b, s, :],
output row = concat(v, flat(v (x) v), flat(v (x) v (x) v)) -> [8, 32, 266304].

Sharding: pure data-parallel over batch; core c computes x[c] -> [32, 266304].

Per-core dataflow (segments in pairs a=2p, b=2p+1, fully per-pair pipelined):
  mmA (K=2, f32): psA[128,64] = lhsT_pad_p.T @ vcomp_p
      rows 0:64 = v_a (x) v_a (level2 of seg a), rows 64:128 = v_b (x) v_b.
  reshape: psA -> SBUF -> bf16 cast -> DRAM scratch l2bf_dram[a:a+2, :]
      (partition-major iteration == the flattened level2), read straight
      back as pair_bf[2, 4096] (seg per partition, 8KB contiguous rows).
  mmB (K=2, single bf16 matmul, x8 chunks): psB[128,512] =
      lhsT_bf[:,p*128:+128].T @ pair_bf chunk, so rows 0:64 =
      v_a[i] * l2flat_a[chunk c] (level3), rows 64:128 seg b.
      Error ~2^-8 (bf16 rounding of both factors; elements are pure products
      so no cancellation) vs 2e-2 tolerance.
  PSUM->SBUF copies (DVE/ACT alternating) into outsb[128, 4096], then two
      1MB flat-dest DMAs per pair write the level3 blocks (64 x 16KB each;
      flat dests spray across all 16 SDMA engines, 2-row dests do not).
  level2 output: one batched DRAM->DRAM cast DMA (bf16 -> f32) at the end
      of issue order; executes while mmB stream drains.

Measured (NTFF profile, core 0): ~137us vs ~212us for the previous
hi/lo-compensated octet kernel; HBM write wall is ~319 GB/s/NC so the
33.5MB/core output stream floors at ~105us + ~15us ramp/drain.
"""

import numpy as np

B, S, D = 8, 32, 64
PAIRS = S // 2        # 16
D2 = D * D            # 4096
D3 = D2 * D           # 262144
ROW = D + D2 + D3     # 266304
L2OFF = D             # 64
L3OFF = D + D2        # 4160
NCHUNK = 8            # D2 / 512 psum-bank chunks
CHUNK = D2 // NCHUNK  # 512

_compiled = None


def _build(big_bufs=7, l3_engines=("scalar", "sync"), l3_split=2,
           copy_eng="alt", scratch_eng="gpsimd", l2cast_eng="gpsimd",
           psb_bufs=4, pair_bufs=8, reshape="dram", resh_eng="sync",
           spool_bufs=8, psa_bufs=3, early_pairs=1, late_pairs=0,
           direct_cast=False, pair0_sb2sb=False, pair0_w=False):
    import concourse.bacc as bacc
    import concourse.mybir as mybir
    import concourse.tile as tile

    f32 = mybir.dt.float32
    bf16 = mybir.dt.bfloat16

    nc = bacc.Bacc("TRN2", target_bir_lowering=False, debug=False)
    x = nc.dram_tensor("x", [S, D], f32, kind="ExternalInput").ap()
    out = nc.dram_tensor("out", [S, ROW], f32, kind="ExternalOutput").ap()

    with tile.TileContext(nc) as tc:
        with (
            tc.tile_pool(name="const", bufs=1) as cpool,
            tc.tile_pool(name="small", bufs=spool_bufs) as spool,
            tc.tile_pool(name="pair", bufs=pair_bufs) as ppool,
            tc.tile_pool(name="big", bufs=big_bufs) as bpool,
            tc.tile_pool(name="dram", bufs=1, space="DRAM") as dpool,
            tc.tile_pool(name="psA", bufs=psa_bufs, space="PSUM") as psa_pool,
            tc.tile_pool(name="psB", bufs=psb_bufs, space="PSUM") as psb_pool,
        ):
            # ---- prologue: load x, negate, build packed v layouts ----
            x_s = cpool.tile([S, D], f32)
            nc.sync.dma_start(out=x_s[:], in_=x[:])
            v_s = cpool.tile([S, D], f32)
            nc.scalar.mul(v_s[:], x_s[:], -1.0)

            # level1 output: out[s, 0:64] = v_s
            nc.sync.dma_start(out=out[:, 0:D], in_=v_s[:])

            # mmA/mmB weights: lhsT_pad[0, p*128 : +64] = v_{2p}
            #                  lhsT_pad[1, p*128+64 : +64] = v_{2p+1}
            lhsT_pad = cpool.tile([2, PAIRS * 128], f32)
            nc.vector.memset(lhsT_pad[:], 0.0)
            dst0 = lhsT_pad[0:1, :].rearrange("p (n c) -> p n c", c=128)[:, :, 0:D]
            nc.sync.dma_start(out=dst0, in_=v_s[0:S:2, :])
            dst1 = lhsT_pad[1:2, :].rearrange("p (n c) -> p n c", c=128)[:, :, D:128]
            nc.sync.dma_start(out=dst1, in_=v_s[1:S:2, :])
            # bf16 twin for mmB (zeros cast to zeros)
            lhsT_bf = cpool.tile([2, PAIRS * 128], bf16)
            nc.vector.tensor_copy(lhsT_bf[:], lhsT_pad[:])

            # mmA moving: v_comp[e, p*64:(p+1)*64] = v_{2p+e}
            v_comp = cpool.tile([2, PAIRS * D], f32)
            nc.sync.dma_start(out=v_comp[0:1, :], in_=v_s[0:S:2, :])
            nc.sync.dma_start(out=v_comp[1:2, :], in_=v_s[1:S:2, :])

            # pair-0 dedicated weights: off the big lhsT_pad build chain,
            # so the ramp-critical first psA starts ~2-3us earlier
            if pair0_w:
                lhsT0 = cpool.tile([2, 128], f32, name="lhsT0")
                nc.gpsimd.memset(lhsT0[:], 0.0)
                nc.sync.dma_start(out=lhsT0[0:1, 0:D], in_=v_s[0:1, :])
                nc.sync.dma_start(out=lhsT0[1:2, D:128], in_=v_s[1:2, :])
                vc0 = cpool.tile([2, D], f32, name="vc0")
                nc.sync.dma_start(out=vc0[0:1, :], in_=v_s[0:1, :])
                nc.sync.dma_start(out=vc0[1:2, :], in_=v_s[1:2, :])
                lhsT0_bf = cpool.tile([2, 128], bf16, name="lhsT0_bf")
                nc.vector.tensor_copy(lhsT0_bf[:], lhsT0[:])

            l2bf_dram = (dpool.tile([S, D2], bf16, name="l2bf_dram")
                         if reshape == "dram" else None)
            pair0_bf = (cpool.tile([2, D2], bf16, name="pair0_bf")
                        if pair0_sb2sb else None)
            # rotating 16-partition window: pair p uses rows 2*(p%8)..+2.
            # Dest partitions 0-15 spread the SBUF->SBUF reshape DMAs over
            # SDMA engines 0/2/4/6; 8 windows of slack before WAR reuse.
            win_bf = (cpool.tile([16, D2], bf16, name="win_bf")
                      if reshape == "sb2sb" else None)

            dma_i = [0]

            def next_eng():
                e = getattr(nc, l3_engines[dma_i[0] % len(l3_engines)])
                dma_i[0] += 1
                return e

            for p in range(PAIRS):
                a, b = 2 * p, 2 * p + 1
                # ---- level2 for this pair ----
                psA = psa_pool.tile([128, D], f32)
                if pair0_w and p == 0:
                    nc.tensor.matmul(psA[:], lhsT0[:], vc0[:],
                                     start=True, stop=True)
                else:
                    nc.tensor.matmul(
                        psA[:],
                        lhsT_pad[:, p * 128:(p + 1) * 128],
                        v_comp[:, p * D:(p + 1) * D],
                        start=True, stop=True,
                    )
                l2bf = spool.tile([128, D], bf16, tag="l2bf")
                if direct_cast:
                    # PSUM -> SBUF with bf16 cast in one DVE op
                    nc.vector.tensor_copy(l2bf[:], psA[:])
                else:
                    l2sb = spool.tile([128, D], f32)
                    nc.scalar.copy(l2sb[:], psA[:])
                    nc.vector.tensor_copy(l2bf[:], l2sb[:])
                if pair0_sb2sb and p == 0:
                    # single-hop reshape for the ramp-critical first pair
                    pair_bf = pair0_bf[:]
                    nc.sync.dma_start(out=pair_bf, in_=l2bf[:])
                    # still write scratch so the batched level2 cast works
                    getattr(nc, scratch_eng).dma_start(
                        out=l2bf_dram[a:a + 2, :], in_=l2bf[:])
                elif reshape == "dram":
                    getattr(nc, scratch_eng).dma_start(
                        out=l2bf_dram[a:a + 2, :], in_=l2bf[:])
                    # read straight back: seg-per-partition rows, 8KB runs
                    pair_bf = ppool.tile([2, D2], bf16)
                    getattr(nc, scratch_eng).dma_start(
                        out=pair_bf[:], in_=l2bf_dram[a:a + 2, :])
                else:
                    w = 2 * (p % 8)
                    pair_bf = win_bf[w:w + 2, :]
                    getattr(nc, resh_eng).dma_start(out=pair_bf, in_=l2bf[:])
                    # level2 output rows, cast bf16 -> f32 during DMA
                    getattr(nc, l2cast_eng).dma_start(
                        out=out[a:a + 2, L2OFF:L3OFF], in_=pair_bf)

                # ---- level3 for this pair (K=2 bf16 matmul) ----
                wp = (lhsT0_bf[:] if (pair0_w and p == 0)
                      else lhsT_bf[:, p * 128:(p + 1) * 128])
                outsb = bpool.tile([128, D2], f32)
                for c in range(NCHUNK):
                    psB = psb_pool.tile([128, CHUNK], f32)
                    nc.tensor.matmul(
                        psB[:], wp,
                        pair_bf[:, c * CHUNK:(c + 1) * CHUNK],
                        start=True, stop=True,
                    )
                    dst = outsb[:, c * CHUNK:(c + 1) * CHUNK]
                    use_dve = (c % 2 == 0) if copy_eng == "alt" else (
                        copy_eng == "dve")
                    if use_dve:
                        nc.vector.tensor_copy(dst, psB[:])
                    else:
                        nc.scalar.copy(dst, psB[:])

                # both segments' level3: flat per-row dests, 64 x 16KB each
                if p < early_pairs or p >= PAIRS - late_pairs:
                    # ramp trim: stream this pair's output per chunk so the
                    # write queues start draining ~5us earlier
                    ov = out[a, L3OFF:ROW].rearrange("(i m) -> i m", m=D2)
                    ovb = out[b, L3OFF:ROW].rearrange("(i m) -> i m", m=D2)
                    for c in range(NCHUNK):
                        cs = slice(c * CHUNK, (c + 1) * CHUNK)
                        next_eng().dma_start(
                            out=ov[:, cs], in_=outsb[0:64, cs])
                        next_eng().dma_start(
                            out=ovb[:, cs], in_=outsb[64:128, cs])
                elif l3_split == 2:
                    next_eng().dma_start(
                        out=out[a, L3OFF:ROW], in_=outsb[0:64, :])
                    next_eng().dma_start(
                        out=out[b, L3OFF:ROW], in_=outsb[64:128, :])
                else:
                    # per-segment halves, contiguous dests (4 DMAs)
                    H = 32 * D2
                    for row, base in ((a, 0), (b, 64)):
                        next_eng().dma_start(
                            out=out[row, L3OFF:L3OFF + H],
                            in_=outsb[base:base + 32, :])
                        next_eng().dma_start(
                            out=out[row, L3OFF + H:ROW],
                            in_=outsb[base + 32:base + 64, :])

            if reshape == "dram":
                # level2 output, all segments: DRAM->DRAM cast bf16 -> f32,
                # overlaps the remaining mmB/DMA stream
                getattr(nc, l2cast_eng).dma_start(
                    out=out[:, L2OFF:L3OFF], in_=l2bf_dram[:])

    nc.compile()
    return nc


def _get_compiled():
    global _compiled
    if _compiled is None:
        _compiled = _build()
    return _compiled


def kernel(x: np.ndarray) -> np.ndarray:
    from concourse.bass_utils import run_bass_kernel_spmd

    assert x.shape == (B, S, D), x.shape
    nc = _get_compiled()
    x = np.ascontiguousarray(x, dtype=np.float32)
    in_maps = [{"x": x[c]} for c in range(B)]
    res = run_bass_kernel_spmd(nc, in_maps, list(range(B)))
    return np.stack([res.results[c]["out"] for c in range(B)], axis=0)
